# revision 2
# baseline (speedup 1.0000x reference)
"""Trainium2 Bass kernel for the ExemplarModel (Mahalanobis-kNN attention).

Reference math (N=1024 queries, M=50000 exemplars, D=512, C=10 classes):
    dist[n,m]  = sum_d Sigma_inv[d] * (x[n,d] - e[m,d])^2
    att[n,m]   = exp(-beta * dist[n,m])
    logits[n,c]= segment_sum(att over exemplars with label c)
    out        = softmax(gamma * logits, axis=1)

Distribution: exemplars/labels sharded along M across 8 NeuronCores
(6250 each, zero-padded to 6272 = 49*128); x, Sigma_inv, beta replicated.
Each core computes partial per-class logits
    P[c,n] = sum_m onehot[m,c] * exp(2*beta*cross[n,m] - beta*e_sq[m])
with cross[m,n] = sum_d e[m,d] * (x*Sigma_inv)[n,d].

v3 design (from trace analysis of the 72.4us v2):
  - measured v2 window: [first MEMSET 5958 -> last teardown inst 79139];
    the walrus semaphore-teardown tail (~10.2us) and ~1.3us of preamble
    are fixed; everything else is user time.
  - cross matmuls (196 fp8-DoubleRow, 218ns cadence) are at the fp8 PE
    peak (41.8us/core floor) — unchanged.
  - segment-sum matmuls moved OFF the DoubleRow path: v2 interleaved 50
    DR matmuls (~10.7us of PE). v3 batches them at the END as 4-way
    column-tiled matmuls (tile_position=(0,32q), tile_size=(128,32)):
    4 independent 128-contraction streams run concurrently in the four
    column groups of the PE array (~5.5us). Each quadrant q accumulates
    the partial logits of tiles {t : t%4==q} at PSUM partitions
    32q..32q+15; the host sums the 4 stripes (and the 8 cores).
    Batching at the end avoids per-transition tiling-mode drains.
  - att tiles for ALL 49 tiles are buffered in SBUF (6.3MB) — exp runs
    on DVE (even tiles, Schraudolph-bits-to-u8 trick) and ACT (odd
    tiles, exact Exp) overlapped with the cross phase exactly as in v2.
  - epilogue: one wide [112,1024] f32->bf16 PSUM->SBUF copy split
    across DVE/ACT by n-half, then 2 DMAs on 2 queues. Replaces v2's
    partition-starved [10,512] copies + 40KB DMA.
  - warmup matmul count trimmed 80 -> 48: v2's 80 warmups ended ~1.8us
    after the input DMAs had already landed; the PE clock is ramped
    after ~3us of continuous work, so fewer bridge matmuls start the
    real cross phase earlier.

The host combines: logits[n,c] = exp(-beta*x_sq[n]) * sum_cores sum_q P,
then gamma + softmax on the tiny [1024,10] result.
"""

import numpy as np
import ml_dtypes

import concourse.bass as bass
import concourse.bacc as bacc
import concourse.tile as tile
from concourse import mybir
from concourse import bass_utils

# Problem constants (hardcoded per contract; kernel.py must be self-contained).
N = 1024          # queries
M = 50000         # exemplars (global)
D = 512           # feature dim
C = 10            # classes
N_CORES = 8
M_LOC = M // N_CORES          # 6250 exemplars per core
P = 128                       # partitions
T_TILES = (M_LOC + P - 1) // P  # 49 tiles per core
M_PAD = T_TILES * P           # 6272
KC = D // P                   # 4 contraction chunks
CP = 16                       # one-hot pitch
NH = N // 512                 # 2 matmul free-dim halves
NQ = 4                        # column-tile quadrants for the segment phase
OUT_P = 32 * (NQ - 1) + CP    # output rows shipped to host (incl. gaps)
N_WARM = 48                   # PE warmup matmuls during DMA fill

LOG2E = float(np.log2(np.e))
DELTA = -0.46                 # Schraudolph magic offset for e4m3 (tuned)

FP32 = mybir.dt.float32
BF16 = mybir.dt.bfloat16
FP8 = mybir.dt.float8e4
U8 = mybir.dt.uint8
NP_FP8 = ml_dtypes.float8_e4m3


def build_nc(t_tiles=T_TILES, n=N, debug=False):
    """Build the per-core Bass program (SPMD: same program, per-core data)."""
    nc = bacc.Bacc("TRN2", target_bir_lowering=False, debug=debug,
                   num_devices=N_CORES)
    nh = n // 512

    eTt_dram = nc.dram_tensor("eTt", [P, t_tiles * D], FP8, kind="ExternalInput")
    # xsT pre-packed host-side into [p, (k, n)]
    xsT_dram = nc.dram_tensor("xsT", [P, (D // P) * n], FP8,
                              kind="ExternalInput")
    w_dram = nc.dram_tensor("w", [P, t_tiles * CP], FP8, kind="ExternalInput")
    # cb = [ba | bd | sc] packed: one DMA covers every f32 constant
    cb_dram = nc.dram_tensor("cb", [P, 2 * t_tiles + 2], FP32,
                             kind="ExternalInput")
    out_dram = nc.dram_tensor("out", [OUT_P, n], BF16, kind="ExternalOutput")

    with tile.TileContext(nc) as tc:
        with (
            tc.tile_pool(name="const", bufs=1) as const_pool,
            tc.tile_pool(name="crossp", bufs=3, space="PSUM") as cross_pool,
            tc.tile_pool(name="logitp", bufs=1, space="PSUM") as logit_pool,
        ):
            # ---- one-time preamble ----
            # Scalar-queue order is latency-driven: the first cross matmul
            # needs xsT pair-chunk 0, then the act constants, then w.
            xsT_p0 = const_pool.tile([P, 2 * n], FP8, tag="xsTp0")
            xsT_p1 = const_pool.tile([P, 2 * n], FP8, tag="xsTp1")
            ba = const_pool.tile([P, t_tiles], FP32, tag="ba")
            bd = const_pool.tile([P, t_tiles], FP32, tag="bd")
            # per-engine scale constants in SEPARATE tiles (shared-tile
            # scalar operands slow the act engines)
            sc_a = const_pool.tile([P, 1], FP32, tag="sca")
            sc_d = const_pool.tile([P, 1], FP32, tag="scd")
            w_f8 = const_pool.tile([P, t_tiles * CP], FP8, tag="w8")
            # p0 on the scalar queue, p1 on the sync queue (after the tiny
            # eT g0) so both xsT halves stream concurrently at the cold
            # start instead of serially.
            nc.scalar.dma_start(xsT_p0[:], xsT_dram[:, 0:2 * n])
            nc.scalar.dma_start(sc_a[:], cb_dram[:, 2 * t_tiles:2 * t_tiles + 1])
            nc.scalar.dma_start(sc_d[:], cb_dram[:, 2 * t_tiles + 1:2 * t_tiles + 2])
            nc.scalar.dma_start(ba[:], cb_dram[:, 0:t_tiles])
            nc.scalar.dma_start(bd[:], cb_dram[:, t_tiles:2 * t_tiles])
            nc.scalar.dma_start(w_f8[:], w_dram[:])
            xsT_qap = [[t_[:].rearrange("p (k n) -> p k n", n=n)
                        [:, :, h * 512:(h + 1) * 512] for h in range(nh)]
                       for t_ in (xsT_p0, xsT_p1)]

            # Tiled exemplar loads on the Sync HWDGE queue: graded group
            # sizes — small first groups so the early tiles land with low
            # latency, big groups later for issue/semaphore efficiency.
            group_sizes = [1, 2, 2, 2, 4, 8]
            while sum(group_sizes) + 8 <= t_tiles:
                group_sizes.append(8)
            rem = t_tiles - sum(group_sizes)
            if rem:
                group_sizes.append(rem)
            eT_groups = []
            tile2group = []
            off = 0
            for g, gt in enumerate(group_sizes):
                tile_g = const_pool.tile([P, gt * D], FP8, tag=f"eT{g}")
                nc.sync.dma_start(
                    tile_g[:], eTt_dram[:, off * D:(off + gt) * D])
                if g == 0:
                    nc.sync.dma_start(xsT_p1[:], xsT_dram[:, 2 * n:4 * n])
                for lo in range(gt):
                    tile2group.append((g, lo))
                eT_groups.append(tile_g)
                off += gt

            # Full-width PSUM tile: the 4 column-tile quadrants accumulate
            # partial logits at partitions {32q .. 32q+15}; the warmup
            # matmuls scribble on partitions 0-31 first (each quadrant's
            # start=True matmul resets its own region afterwards).
            logits_full = logit_pool.tile([P, n], FP32)

            # PE warmup: narrow DR matmuls on a zeroed scratch tile to start
            # the clock ramp while the first DMAs land (the DVFS clock
            # decays within ~1us of idle; full rate after ~3us busy).
            scratch = const_pool.tile([P, 2 * P], FP8, tag="scr")
            nc.gpsimd.memset(scratch[:], 0)
            scr_pairs = scratch[:].rearrange("p (i n) -> p i n", i=2)
            for _ in range(N_WARM):
                nc.tensor.matmul(
                    logits_full[:32, :64], lhsT=scr_pairs[:, :, :32],
                    rhs=scr_pairs[:, :, :64], start=True, stop=True,
                    perf_mode=mybir.MatmulPerfMode.DoubleRow,
                    skip_group_check=True)

            # ---- cross + exp over all exemplar tiles ----
            # att tiles are all buffered in SBUF; the segment phase runs
            # afterwards in one column-tiled block.
            att_tiles = []
            for t in range(t_tiles):
                g, lo = tile2group[t]
                eT_t = eT_groups[g][:, lo * D:(lo + 1) * D].rearrange(
                    "p (k m) -> p k m", m=P)

                # cross[m, n] = sum_d e[m,d] * xs[n,d]
                # fp8 DoubleRow: each matmul consumes a pair of 128-d chunks
                cross_ps = cross_pool.tile([P, n], FP32, tag="cross")
                for j in range(KC // 2):
                    for h in range(nh):
                        nc.tensor.matmul(
                            cross_ps[:, h * 512:(h + 1) * 512],
                            lhsT=eT_t[:, 2 * j:2 * j + 2, :],
                            rhs=xsT_qap[j][h],
                            start=(j == 0), stop=(j == KC // 2 - 1),
                            perf_mode=mybir.MatmulPerfMode.DoubleRow)

                # att = exp(2*beta*cross - beta*e_sq), alternating engines:
                # even tiles Schraudolph bits on DVE (uint8 out, bitcast
                # fp8), odd tiles exact Exp on ScalarE (fp8 out).
                att_t = const_pool.tile([P, n], FP8, tag=f"att{t}")
                att_tiles.append(att_t)
                if t == t_tiles - 1:
                    # last tile: split exp across BOTH engines by n-half so
                    # its att is ready as early as possible
                    nc.vector.tensor_scalar(
                        att_t[:, :512].bitcast(U8), cross_ps[:, :512],
                        sc_d[:], bd[:, t:t + 1],
                        mybir.AluOpType.mult, mybir.AluOpType.add)
                    nc.scalar.activation(att_t[:, 512:], cross_ps[:, 512:],
                                         mybir.ActivationFunctionType.Exp,
                                         bias=ba[:, t:t + 1],
                                         scale=sc_a[:])
                elif t % 2 == 0:
                    nc.vector.tensor_scalar(
                        att_t[:].bitcast(U8), cross_ps[:],
                        sc_d[:], bd[:, t:t + 1],
                        mybir.AluOpType.mult, mybir.AluOpType.add)
                else:
                    nc.scalar.activation(att_t[:], cross_ps[:],
                                         mybir.ActivationFunctionType.Exp,
                                         bias=ba[:, t:t + 1],
                                         scale=sc_a[:])

            # ---- batched column-tiled segment phase ----
            # Quadrant q owns tiles {q, q+4, ...}; its matmuls accumulate
            # at PSUM partitions 32q..32q+15 in column group q of the PE
            # array. The 4 groups stream concurrently (no DoubleRow —
            # column tiling and Double-FP8 are mutually exclusive).
            q_last = {q: max(t for t in range(t_tiles) if t % NQ == q)
                      for q in range(NQ)}
            n_groups_seg = (t_tiles + NQ - 1) // NQ
            for g in range(n_groups_seg):
                for q in range(NQ):
                    t = g * NQ + q
                    if t >= t_tiles:
                        continue
                    att_t = att_tiles[t]
                    w_t = w_f8[:, t * CP:(t + 1) * CP]
                    for h in range(nh):
                        nc.tensor.matmul(
                            logits_full[32 * q:32 * q + CP,
                                        h * 512:(h + 1) * 512],
                            lhsT=w_t,
                            rhs=att_t[:, h * 512:(h + 1) * 512],
                            start=(g == 0), stop=(t == q_last[q]),
                            tile_position=(0, 32 * q),
                            skip_group_check=True)

            # ---- epilogue: wide bf16 copy + 2 DMAs ----
            out_sb = const_pool.tile([OUT_P, n], BF16, tag="out")
            nc.vector.tensor_copy(out_sb[:, :512], logits_full[:OUT_P, :512])
            nc.scalar.copy(out_sb[:, 512:], logits_full[:OUT_P, 512:])
            nc.sync.dma_start(out_dram[:, :512], out_sb[:, :512])
            nc.scalar.dma_start(out_dram[:, 512:], out_sb[:, 512:])

    nc.compile()
    return nc


def make_in_maps(x, exemplars, labels, Sigma_inv, beta, gamma,
                 t_tiles=T_TILES):
    """Shard the full inputs into per-core in_maps (host-side glue)."""
    x = np.asarray(x, dtype=np.float32)
    exemplars = np.asarray(exemplars, dtype=np.float32)
    labels = np.asarray(labels).astype(np.int64)
    Sigma_inv = np.asarray(Sigma_inv, dtype=np.float32)
    beta = float(np.asarray(beta).reshape(-1)[0])

    m_pad = t_tiles * P
    # xsT packed to device layout [p, (k, n)]: xsT[p, k*N+n] = xs[k*128+p, n]
    xsT = np.ascontiguousarray((x * Sigma_inv).T).astype(NP_FP8)  # [D, N]
    xsT = np.ascontiguousarray(
        xsT.reshape(KC, P, N).transpose(1, 0, 2).reshape(P, KC * N))
    e_sq_full = np.einsum("md,d->m", exemplars * exemplars, Sigma_inv)

    m_loc = M // N_CORES
    in_maps = []
    for c in range(N_CORES):
        e_shard = np.zeros((m_pad, D), dtype=np.float32)
        e_shard[:m_loc] = exemplars[c * m_loc:(c + 1) * m_loc]
        # eTt[p, t*512 + k*128 + m] = e_shard[t*128 + m, k*128 + p]
        eTt = np.ascontiguousarray(
            e_shard.reshape(t_tiles, P, KC, P).transpose(3, 0, 2, 1)
            .reshape(P, t_tiles * D)).astype(NP_FP8)
        lab = labels[c * m_loc:(c + 1) * m_loc]
        onehot = np.zeros((m_pad, CP), dtype=np.float32)
        onehot[np.arange(m_loc), lab] = 1.0
        w_packed = np.ascontiguousarray(
            onehot.reshape(t_tiles, P, CP).transpose(1, 0, 2)
            .reshape(P, t_tiles * CP)).astype(NP_FP8)
        esq = np.zeros(m_pad, dtype=np.float32)
        esq[:m_loc] = e_sq_full[c * m_loc:(c + 1) * m_loc]
        esq_t = esq.reshape(t_tiles, P).T          # [P, t_tiles]
        cb = np.zeros((P, 2 * t_tiles + 2), dtype=np.float32)
        cb[:, 0:t_tiles] = -beta * esq_t
        cb[:, t_tiles:2 * t_tiles] = 56.0 + DELTA - 8.0 * LOG2E * beta * esq_t
        cb[:, 2 * t_tiles] = 2.0 * beta
        cb[:, 2 * t_tiles + 1] = 16.0 * beta * LOG2E
        in_maps.append({
            "eTt": eTt, "xsT": xsT, "w": w_packed, "cb": cb,
        })
    return in_maps


def partial_logits(core_outs):
    """Sum the per-core quadrant stripes into the [C, N] partial logits."""
    total = np.zeros((C, N), dtype=np.float32)
    for o in core_outs:
        o = np.asarray(o, dtype=np.float32)       # [OUT_P, N] from bf16
        for q in range(NQ):
            total += o[32 * q:32 * q + C]
    return total


def finalize(core_outs, x, Sigma_inv, beta, gamma):
    """Combine per-core partial logits into the full softmax output."""
    x = np.asarray(x, dtype=np.float32)
    Sigma_inv = np.asarray(Sigma_inv, dtype=np.float32)
    beta = float(np.asarray(beta).reshape(-1)[0])
    gamma = float(np.asarray(gamma).reshape(-1)[0])

    partial = partial_logits(core_outs)                   # [C, N]
    x_sq = np.einsum("nd,d->n", x * x, Sigma_inv)         # [N]
    logits = np.exp(-beta * x_sq)[:, None].astype(np.float32) * partial.T
    z = gamma * logits
    z = z - z.max(axis=1, keepdims=True)
    ez = np.exp(z)
    return (ez / ez.sum(axis=1, keepdims=True)).astype(np.float32)


_NC_CACHE = {}


def kernel(x, exemplars, labels, Sigma_inv, beta, gamma):
    if "nc" not in _NC_CACHE:
        _NC_CACHE["nc"] = build_nc()
    nc = _NC_CACHE["nc"]
    in_maps = make_in_maps(x, exemplars, labels, Sigma_inv, beta, gamma)
    res = bass_utils.run_bass_kernel_spmd(nc, in_maps,
                                          core_ids=list(range(N_CORES)))
    core_outs = [r["out"] for r in res.results]
    return finalize(core_outs, x, Sigma_inv, beta, gamma)
